# revision 14
# baseline (speedup 1.0000x reference)
"""Trainium2 Bass kernel for nn_MultiHeadAttention (fused MHA + residual + LayerNorm).

Sharding: 8 cores = 2 batches x 4 query-slices of 512 tokens. Each core:
  - projects Q for its 512-token slice (transposed layout QT [D, SL])
  - projects K (transposed, full seq) and V (natural, full seq) for its batch
    (replicated across the 4 cores of the batch -> zero collectives)
  - computes scores^T per head (k-major) -> exp -> PV matmul -> context^T
  - computes scores per head (q-major) -> exp (accum_out = softmax denom)
    -> normalize -> writes attn probs output
  - output projection + bias + residual + LayerNorm -> y slice

kernel(**inputs) takes FULL inputs, shards on host, runs 8-core SPMD,
reassembles FULL outputs (y, attn).
"""
from contextlib import ExitStack

import numpy as np

import concourse.bass as bass
import concourse.tile as tile
from concourse import bacc, mybir
from concourse.masks import make_identity
from concourse.bass_utils import run_bass_kernel_spmd

F32 = mybir.dt.float32
AF = mybir.ActivationFunctionType
ALU = mybir.AluOpType

# Full-size problem constants (hardcoded per harness contract)
D_MODEL = 1024
SEQ = 2048
N_HEADS = 16
D_K = 64
LN_EPS = 1e-5
N_CORES = 8


def _chunks3(n):
    out, i = [], 0
    while i < n:
        out.append(list(range(i, min(i + 3, n))))
        i += 3
    return out


def build_nc(D=D_MODEL, S=SEQ, SL=None, H=N_HEADS, eps=LN_EPS):
    """Build the per-core SPMD program. SL = this core's query-slice length."""
    if SL is None:
        SL = S // 4
    DK = D // H
    assert DK == 64, "head packing assumes d_k == 64"
    KC = D // 128   # contraction chunks over d_model
    KB = S // 128   # key blocks
    QB = SL // 128  # query blocks in slice
    scale = 1.0 / float(np.sqrt(DK))

    nc = bacc.Bacc("TRN2", debug=False)

    xT = nc.dram_tensor("xT", [D, S], F32, kind="ExternalInput")
    xTq = nc.dram_tensor("xTq", [D, SL], F32, kind="ExternalInput")
    xrow = nc.dram_tensor("xrow", [SL, D], F32, kind="ExternalInput")
    wqT = nc.dram_tensor("wqT", [D, D], F32, kind="ExternalInput")
    wkT = nc.dram_tensor("wkT", [D, D], F32, kind="ExternalInput")
    wvT = nc.dram_tensor("wvT", [D, D], F32, kind="ExternalInput")
    woT = nc.dram_tensor("woT", [D, D], F32, kind="ExternalInput")
    bo = nc.dram_tensor("bo", [1, D], F32, kind="ExternalInput")
    gam = nc.dram_tensor("gam", [1, D], F32, kind="ExternalInput")
    bet = nc.dram_tensor("bet", [1, D], F32, kind="ExternalInput")
    attn = nc.dram_tensor("attn", [H, SL, S], F32, kind="ExternalOutput")
    y = nc.dram_tensor("y", [SL, D], F32, kind="ExternalOutput")

    with tile.TileContext(nc) as tc, ExitStack() as ctx:
        pers = ctx.enter_context(tc.tile_pool(name="pers", bufs=1))
        dram = ctx.enter_context(tc.tile_pool(name="dram", bufs=1, space="DRAM"))

        ident = pers.tile([128, 128], F32)
        make_identity(nc, ident)
        eps_sb = pers.tile([128, 1], F32)
        nc.vector.memset(eps_sb, eps)

        QT_sb = pers.tile([128, KC, SL], F32)     # Q^T, d on partitions
        KT_sb = pers.tile([128, KC, S], F32)      # K^T, d on partitions
        vstage = dram.tile([S, D], F32)

        # ---------------- P1: projections ----------------
        with tc.tile_pool(name="xp", bufs=1) as xp, \
             tc.tile_pool(name="wmp", bufs=2) as wmp, \
             tc.tile_pool(name="wnp", bufs=1) as wnp, \
             tc.tile_pool(name="p1w", bufs=2) as p1w, \
             tc.tile_pool(name="psP", bufs=2, space="PSUM") as psP:
            xT_sb = xp.tile([128, KC, S], F32)
            xTq_sb = xp.tile([128, KC, SL], F32)
            xT_r = xT.ap().rearrange("(ko ki) s -> ki ko s", ki=128)
            for k in range(KC):
                nc.sync.dma_start(xT_sb[:, k, :], xT_r[:, k, :])
            nc.sync.dma_start(
                xTq_sb[:, :, :], xTq.ap().rearrange("(ko ki) s -> ki ko s", ki=128)
            )
            wqT_r = wqT.ap().rearrange("(ko ki) d -> ki ko d", ki=128)
            wkT_r = wkT.ap().rearrange("(ko ki) d -> ki ko d", ki=128)
            wvT_r = wvT.ap().rearrange("(ko ki) d -> ki ko d", ki=128)

            # Q^T = wqT.T @ xTq ; K^T = wkT.T @ xT  (weights streamed per m)
            for w_r, dst, src, width in (
                (wqT_r, QT_sb, xTq_sb, SL),
                (wkT_r, KT_sb, xT_sb, S),
            ):
                for m in range(KC):
                    w_m = wmp.tile([128, KC, 128], F32, tag="wm")
                    nc.sync.dma_start(
                        w_m[:, :, :], w_r[:, :, m * 128:(m + 1) * 128]
                    )
                    for n0 in range(0, width, 512):
                        nw = min(512, width - n0)
                        ps_full = psP.tile([128, 512], F32, tag="p1", name="p1ps")
                        ps = ps_full[:, :nw]
                        for k in range(KC):
                            nc.tensor.matmul(
                                ps, w_m[:, k, :], src[:, k, n0:n0 + nw],
                                start=(k == 0), stop=(k == KC - 1),
                            )
                        nc.vector.tensor_copy(out=dst[:, m, n0:n0 + nw], in_=ps)

            # V = xT.T @ wvT  (natural layout [tokens, dv]) -> DRAM staging
            for n0 in range(0, D, 256):
                nw = min(256, D - n0)
                w_n = wnp.tile([128, KC, 256], F32, tag="wn")
                nc.sync.dma_start(w_n[:, :, :nw], wvT_r[:, :, n0:n0 + nw])
                for m in range(KB):
                    ps_full = psP.tile([128, 512], F32, tag="p1", name="p1ps")
                    ps = ps_full[:, :nw]
                    for k in range(KC):
                        nc.tensor.matmul(
                            ps, xT_sb[:, k, m * 128:(m + 1) * 128],
                            w_n[:, k, :nw],
                            start=(k == 0), stop=(k == KC - 1),
                        )
                    vrow = p1w.tile([128, 256], F32, tag="vrow")
                    nc.vector.tensor_copy(out=vrow[:, :nw], in_=ps)
                    nc.sync.dma_start(
                        vstage[m * 128:(m + 1) * 128, n0:n0 + nw], vrow[:, :nw]
                    )

        # ctxT/woT live from P2 through P3
        late = ctx.enter_context(tc.tile_pool(name="late", bufs=1))
        ctxT_sb = late.tile([128, KC, SL], F32)   # context^T, dv on partitions
        woT_sb = late.tile([128, KC, D], F32)
        nc.sync.dma_start(
            woT_sb[:, :, :], woT.ap().rearrange("(ko ki) d -> ki ko d", ki=128)
        )

        # ---------------- P2: attention per head ----------------
        vstage_r = vstage[:].rearrange("(ko ki) d -> ki ko d", ki=128)
        kb_batches = _chunks3(KB)
        BMAX = SL * min(3, KB)
        NW = min(512, S)

        with tc.tile_pool(name="p2w", bufs=2) as p2w, \
             tc.tile_pool(name="probs", bufs=3) as probsp, \
             tc.tile_pool(name="psA", bufs=1, space="PSUM") as psA, \
             tc.tile_pool(name="psB", bufs=1, space="PSUM") as psB, \
             tc.tile_pool(name="psC", bufs=1, space="PSUM") as psC:
            for h in range(H):
                pi0 = (h % 2) * 64          # partition offset of this head's d rows
                po = h // 2                 # chunk index of this head's d rows
                tp0 = 64 - pi0              # opposite half, for denom transpose
                QTh = QT_sb[pi0:pi0 + 64, po, :]

                vh = p2w.tile([128, KB, DK], F32, tag="vh")
                nc.sync.dma_start(vh[:, :, :], vstage_r[:, :, h * DK:(h + 1) * DK])

                denom = p2w.tile([128, QB], F32, tag="denom")
                psc = psC.tile([128, 512], F32, tag="ctx")

                # --- B pass: scores^T -> exp -> PV (context^T, unnormalized)
                for kbs in kb_batches:
                    bps_full = psB.tile([128, BMAX], F32, tag="B", name="bps")
                    bps = bps_full[:, :SL * len(kbs)]
                    for i, kb in enumerate(kbs):
                        nc.tensor.matmul(
                            bps[:, SL * i:SL * (i + 1)],
                            KT_sb[pi0:pi0 + 64, po, kb * 128:(kb + 1) * 128],
                            QTh,
                            start=True, stop=True,
                        )
                    pbt_full = probsp.tile([128, BMAX], F32, tag="pbt", name="pbt")
                    pbt = pbt_full[:, :SL * len(kbs)]
                    nc.scalar.activation(out=pbt, in_=bps, func=AF.Exp, scale=scale)
                    for i, kb in enumerate(kbs):
                        nc.tensor.matmul(
                            psc[pi0:pi0 + 64, :SL],
                            vh[:, kb, :],
                            pbt[:, SL * i:SL * (i + 1)],
                            start=(kb == 0), stop=(kb == KB - 1),
                            tile_position=(0, pi0),
                        )

                # --- A pass: scores -> exp (+denom) -> normalize -> attn out
                for qb in range(QB):
                    aps = psA.tile([128, S], F32, tag="A")
                    for c in range(S // NW):
                        nc.tensor.matmul(
                            aps[:, NW * c:NW * (c + 1)],
                            QT_sb[pi0:pi0 + 64, po, qb * 128:(qb + 1) * 128],
                            KT_sb[pi0:pi0 + 64, po, NW * c:NW * (c + 1)],
                            start=True, stop=True,
                        )
                    pa = probsp.tile([128, S], F32, tag="pa")
                    nc.scalar.activation(
                        out=pa, in_=aps, func=AF.Exp, scale=scale,
                        accum_out=denom[:, qb:qb + 1],
                    )
                    rec = p2w.tile([128, QB], F32, tag="rec")
                    nc.vector.reciprocal(rec[:, qb:qb + 1], denom[:, qb:qb + 1])
                    nc.gpsimd.tensor_scalar(
                        pa, pa, rec[:, qb:qb + 1], None, ALU.mult
                    )
                    nc.sync.dma_start(attn[h, qb * 128:(qb + 1) * 128, :], pa)

                # --- context^T scale by 1/denom (q on free dim) and place
                rec = p2w.tile([128, QB], F32, tag="rec2")
                nc.vector.reciprocal(rec[:, :], denom[:, :])
                # replicate each recip column 128x along free dim
                rec_rep = p2w.tile([128, QB, 128], F32, tag="rec_rep")
                nc.vector.tensor_copy(
                    out=rec_rep[:, :, :],
                    in_=rec[:, :, None].to_broadcast((128, QB, 128)),
                )
                # transpose each [128,128] block: every output row holds the
                # per-q recip, landing in the (now idle) A psum region
                aps_t = psA.tile([128, S], F32, tag="A", name="apst")
                for c in range(QB):
                    nc.tensor.transpose(
                        aps_t[:, 128 * c:128 * (c + 1)],
                        rec_rep[:, c, :], ident,
                    )
                Rrep = p2w.tile([128, SL], F32, tag="Rrep")
                nc.vector.tensor_copy(
                    out=Rrep[pi0:pi0 + 64, :], in_=aps_t[pi0:pi0 + 64, :SL]
                )
                nc.vector.tensor_tensor(
                    ctxT_sb[pi0:pi0 + 64, po, :],
                    psc[pi0:pi0 + 64, :SL],
                    Rrep[pi0:pi0 + 64, :],
                    ALU.mult,
                )

        # ---------------- P3: out-proj + residual + LayerNorm ----------------
        with tc.tile_pool(name="p3w", bufs=3) as p3w, \
             tc.tile_pool(name="reps", bufs=1) as repsp, \
             tc.tile_pool(name="psO", bufs=2, space="PSUM") as psO:
            xr_sb = repsp.tile([128, QB, D], F32)
            nc.sync.dma_start(
                xr_sb[:, :, :], xrow.ap().rearrange("(qo qi) d -> qi qo d", qi=128)
            )

            def bcast_load(dr):
                t = repsp.tile([128, D], F32, tag=f"rep_{dr.name}", name=f"rep_{dr.name}")
                src = dr[0:1, :]
                src_b = bass.AP(
                    tensor=src.tensor, offset=src.offset, ap=[[0, 128]] + src.ap[1:]
                )
                nc.gpsimd.dma_start(t[:, :], src_b)
                return t

            bo_rep = bcast_load(bo)
            g_rep = bcast_load(gam)
            b_rep = bcast_load(bet)

            fmax = int(np.gcd(512, D))
            nsub = D // fmax
            for m in range(QB):
                out_t = p3w.tile([128, D], F32, tag="out")
                for n0 in range(0, D, 512):
                    nw = min(512, D - n0)
                    ps_full = psO.tile([128, 512], F32, tag="o", name="ops")
                    ps = ps_full[:, :nw]
                    for k in range(KC):
                        nc.tensor.matmul(
                            ps, ctxT_sb[:, k, m * 128:(m + 1) * 128],
                            woT_sb[:, k, n0:n0 + nw],
                            start=(k == 0), stop=(k == KC - 1),
                        )
                    # out = psum + b_o  (fused copy+bias)
                    nc.vector.tensor_tensor(
                        out_t[:, n0:n0 + nw], ps, bo_rep[:, n0:n0 + nw], ALU.add
                    )
                # residual
                nc.vector.tensor_tensor(
                    out_t[:, :], out_t[:, :], xr_sb[:, m, :], ALU.add
                )
                # LayerNorm
                stats = p3w.tile([128, nsub, 6], F32, tag="st")
                for sg in range(nsub):
                    nc.vector.bn_stats(
                        out=stats[:, sg, :], in_=out_t[:, sg * fmax:(sg + 1) * fmax]
                    )
                mv = p3w.tile([128, 2], F32, tag="mv")
                nc.vector.bn_aggr(out=mv[:, :], in_=stats[:, :, :])
                rstd = p3w.tile([128, 1], F32, tag="rstd")
                nc.scalar.activation(
                    out=rstd[:, :], in_=mv[:, 1:2], func=AF.Sqrt, bias=eps_sb[:, :]
                )
                nc.vector.reciprocal(rstd[:, :], rstd[:, :])
                nc.vector.tensor_scalar(
                    out_t[:, :], out_t[:, :], mv[:, 0:1], rstd[:, :],
                    ALU.subtract, ALU.mult,
                )
                nc.vector.tensor_tensor(out_t[:, :], out_t[:, :], g_rep[:, :], ALU.mult)
                nc.vector.tensor_tensor(out_t[:, :], out_t[:, :], b_rep[:, :], ALU.add)
                nc.sync.dma_start(y[m * 128:(m + 1) * 128, :], out_t[:, :])

    nc.finalize()
    return nc


_NC_CACHE = {}


def _get_nc(key=(D_MODEL, SEQ, N_HEADS)):
    if key not in _NC_CACHE:
        D, S, H = key
        _NC_CACHE[key] = build_nc(D=D, S=S, H=H)
    return _NC_CACHE[key]


def kernel(x, W_q, W_k, W_v, W_o, b_o, ln_gamma, ln_beta):
    x = np.asarray(x, dtype=np.float32)
    B, S, D = x.shape
    H = N_HEADS
    SL = S // 4
    nc = _get_nc((D, S, H))

    wqT = np.ascontiguousarray(np.asarray(W_q, np.float32).T)
    wkT = np.ascontiguousarray(np.asarray(W_k, np.float32).T)
    wvT = np.ascontiguousarray(np.asarray(W_v, np.float32).T)
    woT = np.ascontiguousarray(np.asarray(W_o, np.float32).T)
    bo = np.asarray(b_o, np.float32).reshape(1, D)
    gam = np.asarray(ln_gamma, np.float32).reshape(1, D)
    bet = np.asarray(ln_beta, np.float32).reshape(1, D)

    in_maps = []
    for c in range(N_CORES):
        b, t = c // 4, c % 4
        xT = np.ascontiguousarray(x[b].T)
        in_maps.append({
            "xT": xT,
            "xTq": np.ascontiguousarray(xT[:, SL * t:SL * (t + 1)]),
            "xrow": np.ascontiguousarray(x[b, SL * t:SL * (t + 1), :]),
            "wqT": wqT, "wkT": wkT, "wvT": wvT, "woT": woT,
            "bo": bo, "gam": gam, "bet": bet,
        })

    res = run_bass_kernel_spmd(nc, in_maps, core_ids=list(range(N_CORES)))

    y = np.empty((B, S, D), np.float32)
    attn = np.empty((B, H, S, S), np.float32)
    for c in range(N_CORES):
        b, t = c // 4, c % 4
        y[b, SL * t:SL * (t + 1), :] = res.results[c]["y"]
        attn[b, :, SL * t:SL * (t + 1), :] = res.results[c]["attn"]
    return y, attn


# revision 19
# speedup vs baseline: 1.0710x; 1.0710x over previous
"""Trainium2 Bass kernel for nn_MultiHeadAttention (fused MHA + residual + LayerNorm).

Sharding: 8 cores = 2 batches x 4 query-slices of 512 tokens. Each core:
  - projects Q for its 512-token slice (transposed layout QT [D, SL])
  - projects K (transposed, full seq) and V (natural, full seq) for its batch
    (replicated across the 4 cores of the batch -> zero collectives)
  - computes scores^T per head (k-major) -> exp -> PV matmul -> context^T
  - computes scores per head (q-major) -> exp (accum_out = softmax denom)
    -> normalize -> writes attn probs output
  - output projection + bias + residual + LayerNorm -> y slice

kernel(**inputs) takes FULL inputs, shards on host, runs 8-core SPMD,
reassembles FULL outputs (y, attn).
"""
from contextlib import ExitStack

import numpy as np

import concourse.bass as bass
import concourse.tile as tile
from concourse import bacc, mybir
from concourse.masks import make_identity
from concourse.bass_utils import run_bass_kernel_spmd

F32 = mybir.dt.float32
F32R = mybir.dt.float32r
AF = mybir.ActivationFunctionType
ALU = mybir.AluOpType

# Full-size problem constants (hardcoded per harness contract)
D_MODEL = 1024
SEQ = 2048
N_HEADS = 16
D_K = 64
LN_EPS = 1e-5
N_CORES = 8


def _chunks3(n):
    out, i = [], 0
    while i < n:
        out.append(list(range(i, min(i + 3, n))))
        i += 3
    return out


def build_nc(D=D_MODEL, S=SEQ, SL=None, H=N_HEADS, eps=LN_EPS, trace_sim=False):
    """Build the per-core SPMD program. SL = this core's query-slice length."""
    if SL is None:
        SL = S // 4
    DK = D // H
    assert DK == 64, "head packing assumes d_k == 64"
    KC = D // 128   # contraction chunks over d_model
    KB = S // 128   # key blocks
    QB = SL // 128  # query blocks in slice
    scale = 1.0 / float(np.sqrt(DK))

    nc = bacc.Bacc("TRN2", debug=False)

    xT = nc.dram_tensor("xT", [D, S], F32R, kind="ExternalInput")
    xTq = nc.dram_tensor("xTq", [D, SL], F32R, kind="ExternalInput")
    xrow = nc.dram_tensor("xrow", [SL, D], F32, kind="ExternalInput")
    wqT = nc.dram_tensor("wqT", [D, D], F32R, kind="ExternalInput")
    wkT = nc.dram_tensor("wkT", [D, D], F32R, kind="ExternalInput")
    wvT = nc.dram_tensor("wvT", [D, D], F32R, kind="ExternalInput")
    woT = nc.dram_tensor("woT", [D, D], F32R, kind="ExternalInput")
    bo = nc.dram_tensor("bo", [1, D], F32, kind="ExternalInput")
    gam = nc.dram_tensor("gam", [1, D], F32, kind="ExternalInput")
    bet = nc.dram_tensor("bet", [1, D], F32, kind="ExternalInput")
    attn = nc.dram_tensor("attn", [H, SL, S], F32, kind="ExternalOutput")
    y = nc.dram_tensor("y", [SL, D], F32, kind="ExternalOutput")

    with tile.TileContext(nc, trace_sim=trace_sim) as tc, ExitStack() as ctx:
        pers = ctx.enter_context(tc.tile_pool(name="pers", bufs=1))
        dram = ctx.enter_context(tc.tile_pool(name="dram", bufs=1, space="DRAM"))

        ident = pers.tile([128, 128], F32)
        make_identity(nc, ident)
        eps_sb = pers.tile([128, 1], F32)
        nc.vector.memset(eps_sb, eps)

        QT_sb = pers.tile([128, KC, SL], F32R)     # Q^T, d on partitions
        KT_sb = pers.tile([128, KC, S], F32R)      # K^T, d on partitions
        vstage = dram.tile([S, D], F32R)

        # ---------------- P1: projections ----------------
        with tc.tile_pool(name="xp", bufs=1) as xp, \
             tc.tile_pool(name="wmp", bufs=2) as wmp, \
             tc.tile_pool(name="wnp", bufs=1) as wnp, \
             tc.tile_pool(name="p1w", bufs=2) as p1w, \
             tc.tile_pool(name="psP", bufs=2, space="PSUM") as psP:
            xT_sb = xp.tile([128, KC, S], F32R)
            xTq_sb = xp.tile([128, KC, SL], F32R)
            xT_r = xT.ap().rearrange("(ko ki) s -> ki ko s", ki=128)
            for k in range(KC):
                nc.sync.dma_start(xT_sb[:, k, :], xT_r[:, k, :])
            nc.sync.dma_start(
                xTq_sb[:, :, :], xTq.ap().rearrange("(ko ki) s -> ki ko s", ki=128)
            )
            wqT_r = wqT.ap().rearrange("(ko ki) d -> ki ko d", ki=128)
            wkT_r = wkT.ap().rearrange("(ko ki) d -> ki ko d", ki=128)
            wvT_r = wvT.ap().rearrange("(ko ki) d -> ki ko d", ki=128)

            # Q^T = wqT.T @ xTq ; K^T = wkT.T @ xT  (weights streamed per m)
            for w_r, dst, src, width in (
                (wqT_r, QT_sb, xTq_sb, SL),
                (wkT_r, KT_sb, xT_sb, S),
            ):
                for m in range(KC):
                    w_m = wmp.tile([128, KC, 128], F32R, tag="wm")
                    nc.sync.dma_start(
                        w_m[:, :, :], w_r[:, :, m * 128:(m + 1) * 128]
                    )
                    for n0 in range(0, width, 512):
                        nw = min(512, width - n0)
                        ps_full = psP.tile([128, 512], F32, tag="p1", name="p1ps")
                        ps = ps_full[:, :nw]
                        for k in range(KC):
                            nc.tensor.matmul(
                                ps, (w_m[:, k, :]), (src[:, k, n0:n0 + nw]),
                                start=(k == 0), stop=(k == KC - 1),
                            )
                        nc.vector.tensor_copy(out=dst[:, m, n0:n0 + nw], in_=ps)

            # V = xT.T @ wvT  (natural layout [tokens, dv]) -> DRAM staging
            for n0 in range(0, D, 256):
                nw = min(256, D - n0)
                w_n = wnp.tile([128, KC, 256], F32R, tag="wn")
                nc.sync.dma_start(w_n[:, :, :nw], wvT_r[:, :, n0:n0 + nw])
                for m in range(KB):
                    ps_full = psP.tile([128, 512], F32, tag="p1", name="p1ps")
                    ps = ps_full[:, :nw]
                    for k in range(KC):
                        nc.tensor.matmul(
                            ps, (xT_sb[:, k, m * 128:(m + 1) * 128]),
                            (w_n[:, k, :nw]),
                            start=(k == 0), stop=(k == KC - 1),
                        )
                    vrow = p1w.tile([128, 256], F32R, tag="vrow")
                    nc.vector.tensor_copy(out=vrow[:, :nw], in_=ps)
                    nc.sync.dma_start(
                        vstage[m * 128:(m + 1) * 128, n0:n0 + nw], vrow[:, :nw]
                    )

        # ctxT/woT live from P2 through P3
        late = ctx.enter_context(tc.tile_pool(name="late", bufs=1))
        ctxT_sb = late.tile([128, KC, SL], F32R)   # context^T, dv on partitions
        woT_sb = late.tile([128, KC, D], F32R)
        nc.sync.dma_start(
            woT_sb[:, :, :], woT.ap().rearrange("(ko ki) d -> ki ko d", ki=128)
        )

        # ---------------- P2: attention per head ----------------
        vstage_r = vstage[:].rearrange("(ko ki) d -> ki ko d", ki=128)
        kb_batches = _chunks3(KB)
        BMAX = SL * min(3, KB)
        NW = min(512, S)

        with tc.tile_pool(name="p2w", bufs=2) as p2w, \
             tc.tile_pool(name="probs", bufs=3) as probsp, \
             tc.tile_pool(name="psA", bufs=1, space="PSUM") as psA, \
             tc.tile_pool(name="psB", bufs=1, space="PSUM") as psB, \
             tc.tile_pool(name="psC", bufs=1, space="PSUM") as psC:
            for h in range(H):
                pi0 = (h % 2) * 64          # partition offset of this head's d rows
                po = h // 2                 # chunk index of this head's d rows
                tp0 = 64 - pi0              # opposite half, for denom transpose
                QTh = QT_sb[pi0:pi0 + 64, po, :]

                vh = p2w.tile([128, KB, DK], F32R, tag="vh")
                nc.sync.dma_start(vh[:, :, :], vstage_r[:, :, h * DK:(h + 1) * DK])

                denom = p2w.tile([128, QB], F32, tag="denom")
                psc = psC.tile([128, 512], F32, tag="ctx")

                # --- B pass: scores^T -> exp -> PV (context^T, unnormalized)
                for kbs in kb_batches:
                    bps_full = psB.tile([128, BMAX], F32, tag="B", name="bps")
                    bps = bps_full[:, :SL * len(kbs)]
                    for i, kb in enumerate(kbs):
                        nc.tensor.matmul(
                            bps[:, SL * i:SL * (i + 1)],
                            (KT_sb[pi0:pi0 + 64, po, kb * 128:(kb + 1) * 128]),
                            (QTh),
                            start=True, stop=True,
                        )
                    pbt_full = probsp.tile([128, BMAX], F32R, tag="pbt", name="pbt")
                    pbt = pbt_full[:, :SL * len(kbs)]
                    nc.scalar.activation(out=pbt, in_=bps, func=AF.Exp, scale=scale)
                    for i, kb in enumerate(kbs):
                        nc.tensor.matmul(
                            psc[0:64, :SL],
                            (vh[:, kb, :]),
                            (pbt[:, SL * i:SL * (i + 1)]),
                            start=(kb == 0), stop=(kb == KB - 1),
                        )

                # --- A pass: scores -> exp (+denom) -> normalize -> attn out
                for qb in range(QB):
                    aps = psA.tile([128, S], F32, tag="A")
                    for c in range(S // NW):
                        nc.tensor.matmul(
                            aps[:, NW * c:NW * (c + 1)],
                            (QT_sb[pi0:pi0 + 64, po, qb * 128:(qb + 1) * 128]),
                            (KT_sb[pi0:pi0 + 64, po, NW * c:NW * (c + 1)]),
                            start=True, stop=True,
                        )
                    pa = probsp.tile([128, S], F32, tag="pa")
                    nc.scalar.activation(
                        out=pa, in_=aps, func=AF.Exp, scale=scale,
                        accum_out=denom[:, qb:qb + 1],
                    )
                    rec = p2w.tile([128, QB], F32, tag="rec")
                    nc.vector.reciprocal(rec[:, qb:qb + 1], denom[:, qb:qb + 1])
                    nc.gpsimd.tensor_scalar(
                        pa, pa, rec[:, qb:qb + 1], None, ALU.mult
                    )
                    nc.sync.dma_start(attn[h, qb * 128:(qb + 1) * 128, :], pa)

                # --- context^T scale by 1/denom (q on free dim) and place
                rec = p2w.tile([128, QB], F32, tag="rec2")
                nc.vector.reciprocal(rec[:, :], denom[:, :])
                # replicate each recip column 128x along free dim
                rec_rep = p2w.tile([128, QB, 128], F32, tag="rec_rep")
                nc.vector.tensor_copy(
                    out=rec_rep[:, :, :],
                    in_=rec[:, :, None].to_broadcast((128, QB, 128)),
                )
                # transpose each [128,128] block: every output row holds the
                # per-q recip, landing in the (now idle) A psum region
                aps_t = psA.tile([128, S], F32, tag="A", name="apst")
                for c in range(QB):
                    nc.tensor.transpose(
                        aps_t[:, 128 * c:128 * (c + 1)],
                        rec_rep[:, c, :], ident,
                    )
                Rrep = p2w.tile([128, SL], F32, tag="Rrep")
                nc.vector.tensor_copy(
                    out=Rrep[0:64, :], in_=aps_t[0:64, :SL]
                )
                if pi0 == 0:
                    nc.vector.tensor_tensor(
                        ctxT_sb[0:64, po, :],
                        psc[0:64, :SL],
                        Rrep[0:64, :],
                        ALU.mult,
                    )
                else:
                    cstage = p2w.tile([128, SL], F32R, tag="cstage")
                    nc.vector.tensor_tensor(
                        cstage[0:64, :],
                        psc[0:64, :SL],
                        Rrep[0:64, :],
                        ALU.mult,
                    )
                    nc.sync.dma_start(
                        ctxT_sb[64:128, po, :], cstage[0:64, :]
                    )

        # ---------------- P3: out-proj + residual + LayerNorm ----------------
        with tc.tile_pool(name="p3w", bufs=3) as p3w, \
             tc.tile_pool(name="reps", bufs=1) as repsp, \
             tc.tile_pool(name="psO", bufs=2, space="PSUM") as psO:
            xr_sb = repsp.tile([128, QB, D], F32)
            nc.sync.dma_start(
                xr_sb[:, :, :], xrow.ap().rearrange("(qo qi) d -> qi qo d", qi=128)
            )

            def bcast_load(dr):
                t = repsp.tile([128, D], F32, tag=f"rep_{dr.name}", name=f"rep_{dr.name}")
                src = dr[0:1, :]
                src_b = bass.AP(
                    tensor=src.tensor, offset=src.offset, ap=[[0, 128]] + src.ap[1:]
                )
                nc.gpsimd.dma_start(t[:, :], src_b)
                return t

            bo_rep = bcast_load(bo)
            g_rep = bcast_load(gam)
            b_rep = bcast_load(bet)

            fmax = int(np.gcd(512, D))
            nsub = D // fmax
            for m in range(QB):
                out_t = p3w.tile([128, D], F32, tag="out")
                for n0 in range(0, D, 512):
                    nw = min(512, D - n0)
                    ps_full = psO.tile([128, 512], F32, tag="o", name="ops")
                    ps = ps_full[:, :nw]
                    for k in range(KC):
                        nc.tensor.matmul(
                            ps, (ctxT_sb[:, k, m * 128:(m + 1) * 128]),
                            (woT_sb[:, k, n0:n0 + nw]),
                            start=(k == 0), stop=(k == KC - 1),
                        )
                    # out = psum + b_o  (fused copy+bias)
                    nc.vector.tensor_tensor(
                        out_t[:, n0:n0 + nw], ps, bo_rep[:, n0:n0 + nw], ALU.add
                    )
                # residual
                nc.vector.tensor_tensor(
                    out_t[:, :], out_t[:, :], xr_sb[:, m, :], ALU.add
                )
                # LayerNorm
                stats = p3w.tile([128, nsub, 6], F32, tag="st")
                for sg in range(nsub):
                    nc.vector.bn_stats(
                        out=stats[:, sg, :], in_=out_t[:, sg * fmax:(sg + 1) * fmax]
                    )
                mv = p3w.tile([128, 2], F32, tag="mv")
                nc.vector.bn_aggr(out=mv[:, :], in_=stats[:, :, :])
                rstd = p3w.tile([128, 1], F32, tag="rstd")
                nc.scalar.activation(
                    out=rstd[:, :], in_=mv[:, 1:2], func=AF.Sqrt, bias=eps_sb[:, :]
                )
                nc.vector.reciprocal(rstd[:, :], rstd[:, :])
                nc.vector.tensor_scalar(
                    out_t[:, :], out_t[:, :], mv[:, 0:1], rstd[:, :],
                    ALU.subtract, ALU.mult,
                )
                nc.vector.tensor_tensor(out_t[:, :], out_t[:, :], g_rep[:, :], ALU.mult)
                nc.vector.tensor_tensor(out_t[:, :], out_t[:, :], b_rep[:, :], ALU.add)
                nc.sync.dma_start(y[m * 128:(m + 1) * 128, :], out_t[:, :])

    nc.finalize()
    return nc


_NC_CACHE = {}


def _get_nc(key=(D_MODEL, SEQ, N_HEADS)):
    if key not in _NC_CACHE:
        D, S, H = key
        _NC_CACHE[key] = build_nc(D=D, S=S, H=H)
    return _NC_CACHE[key]


def kernel(x, W_q, W_k, W_v, W_o, b_o, ln_gamma, ln_beta):
    x = np.asarray(x, dtype=np.float32)
    B, S, D = x.shape
    H = N_HEADS
    SL = S // 4
    nc = _get_nc((D, S, H))

    wqT = np.ascontiguousarray(np.asarray(W_q, np.float32).T)
    wkT = np.ascontiguousarray(np.asarray(W_k, np.float32).T)
    wvT = np.ascontiguousarray(np.asarray(W_v, np.float32).T)
    woT = np.ascontiguousarray(np.asarray(W_o, np.float32).T)
    bo = np.asarray(b_o, np.float32).reshape(1, D)
    gam = np.asarray(ln_gamma, np.float32).reshape(1, D)
    bet = np.asarray(ln_beta, np.float32).reshape(1, D)

    in_maps = []
    for c in range(N_CORES):
        b, t = c // 4, c % 4
        xT = np.ascontiguousarray(x[b].T)
        in_maps.append({
            "xT": xT,
            "xTq": np.ascontiguousarray(xT[:, SL * t:SL * (t + 1)]),
            "xrow": np.ascontiguousarray(x[b, SL * t:SL * (t + 1), :]),
            "wqT": wqT, "wkT": wkT, "wvT": wvT, "woT": woT,
            "bo": bo, "gam": gam, "bet": bet,
        })

    res = run_bass_kernel_spmd(nc, in_maps, core_ids=list(range(N_CORES)))

    y = np.empty((B, S, D), np.float32)
    attn = np.empty((B, H, S, S), np.float32)
    for c in range(N_CORES):
        b, t = c // 4, c % 4
        y[b, SL * t:SL * (t + 1), :] = res.results[c]["y"]
        attn[b, :, SL * t:SL * (t + 1), :] = res.results[c]["attn"]
    return y, attn


# revision 32
# speedup vs baseline: 1.5397x; 1.4377x over previous
"""Trainium2 Bass kernel for nn_MultiHeadAttention (fused MHA + residual + LayerNorm).

Sharding: 8 cores = 2 batches x 4 query-slices of 512 tokens. Each core:
  - projects Q for its 512-token slice (transposed layout QT [D, SL])
  - projects K (transposed, full seq) and V (natural, full seq) for its batch
    (replicated across the 4 cores of the batch -> zero collectives)
  - computes scores^T per head (k-major) -> exp -> PV matmul -> context^T
  - computes scores per head (q-major) -> exp (accum_out = softmax denom)
    -> normalize -> writes attn probs output
  - output projection + bias + residual + LayerNorm -> y slice

Matmul inputs use float32r (fast fp32 streaming mode, 1 cycle/row for N>=256).
Pool stack is ordered so attention-phase tiles sit below the projection-phase
x^T buffer: the scheduler can overlap projections with early heads' attention.

kernel(**inputs) takes FULL inputs, shards on host, runs 8-core SPMD,
reassembles FULL outputs (y, attn).
"""
from contextlib import ExitStack

import numpy as np

import concourse.bass as bass
import concourse.tile as tile
from concourse import bacc, mybir
from concourse.masks import make_identity
from concourse.bass_utils import run_bass_kernel_spmd

F32 = mybir.dt.float32
F32R = mybir.dt.float32r
AF = mybir.ActivationFunctionType
ALU = mybir.AluOpType

# Full-size problem constants (hardcoded per harness contract)
D_MODEL = 1024
SEQ = 2048
N_HEADS = 16
D_K = 64
LN_EPS = 1e-5
N_CORES = 8


def _chunks3(n, c=2):
    out, i = [], 0
    while i < n:
        out.append(list(range(i, min(i + c, n))))
        i += c
    return out


def build_nc(D=D_MODEL, S=SEQ, SL=None, H=N_HEADS, eps=LN_EPS, trace_sim=False):
    """Build the per-core SPMD program. SL = this core's query-slice length."""
    if SL is None:
        SL = S // 4
    DK = D // H
    assert DK == 64, "head packing assumes d_k == 64"
    KC = D // 128   # contraction chunks over d_model
    KB = S // 128   # key blocks
    QB = SL // 128  # query blocks in slice
    KB2 = (KB + 1) // 2
    ANW = min(2048, S)    # A-pass psum region width (4 banks)
    NA = S // ANW         # A-pass activates per query block (1)
    scale = 1.0 / float(np.sqrt(DK))

    nc = bacc.Bacc("TRN2", debug=False)

    xT = nc.dram_tensor("xT", [D, S], F32R, kind="ExternalInput")
    xTq = nc.dram_tensor("xTq", [D, SL], F32R, kind="ExternalInput")
    xrow = nc.dram_tensor("xrow", [SL, D], F32, kind="ExternalInput")
    wqT = nc.dram_tensor("wqT", [D, D], F32R, kind="ExternalInput")
    wkT = nc.dram_tensor("wkT", [D, D], F32R, kind="ExternalInput")
    wvT = nc.dram_tensor("wvT", [D, D], F32R, kind="ExternalInput")
    woT = nc.dram_tensor("woT", [D, D], F32R, kind="ExternalInput")
    bo = nc.dram_tensor("bo", [1, D], F32, kind="ExternalInput")
    gam = nc.dram_tensor("gam", [1, D], F32, kind="ExternalInput")
    bet = nc.dram_tensor("bet", [1, D], F32, kind="ExternalInput")
    attn = nc.dram_tensor("attn", [H, SL, S], F32, kind="ExternalOutput")
    y = nc.dram_tensor("y", [SL, D], F32, kind="ExternalOutput")

    with tile.TileContext(nc, trace_sim=trace_sim) as tc, ExitStack() as ctx:
        # ---- program-lifetime pools, attention tiles below x^T on the stack
        pers = ctx.enter_context(tc.tile_pool(name="pers", bufs=1))
        dram = ctx.enter_context(tc.tile_pool(name="dram", bufs=1, space="DRAM"))
        probsp = ctx.enter_context(tc.tile_pool(name="probs", bufs=2))
        pap = ctx.enter_context(tc.tile_pool(name="pap", bufs=2))
        p2w = ctx.enter_context(tc.tile_pool(name="p2w", bufs=2))
        late = ctx.enter_context(tc.tile_pool(name="late", bufs=1))
        rrp = ctx.enter_context(tc.tile_pool(name="rrp", bufs=1))
        psP = ctx.enter_context(tc.tile_pool(name="psP", bufs=1, space="PSUM"))
        psA = ctx.enter_context(tc.tile_pool(name="psA", bufs=1, space="PSUM"))
        psB = ctx.enter_context(tc.tile_pool(name="psB", bufs=1, space="PSUM"))
        psC = ctx.enter_context(tc.tile_pool(name="psC", bufs=1, space="PSUM"))

        ident = pers.tile([128, 128], F32)
        make_identity(nc, ident)
        eps_sb = pers.tile([128, 1], F32)
        nc.vector.memset(eps_sb, eps)

        QT_sb = pers.tile([128, KC, SL], F32R)     # Q^T, d on partitions
        KT_sb = pers.tile([128, KC, S], F32R)      # K^T, d on partitions
        ctxT_sb = late.tile([128, KC, SL], F32R)   # context^T, dv on partitions
        vstages = [dram.tile([S, D // 4], F32R, tag=f"vs{g}", name=f"vs{g}")
                   for g in range(4)]

        wqT_r = wqT.ap().rearrange("(ko ki) d -> ki ko d", ki=128)
        wkT_r = wkT.ap().rearrange("(ko ki) d -> ki ko d", ki=128)
        wvT_r = wvT.ap().rearrange("(ko ki) d -> ki ko d", ki=128)

        # ---------------- P1a: Q projection (small transient x^T slice)
        with tc.tile_pool(name="xpq", bufs=1) as xpq, \
             tc.tile_pool(name="wmq", bufs=2) as wmq:
            xTq_sb = xpq.tile([128, KC, SL], F32R)
            nc.sync.dma_start(
                xTq_sb[:, :, :], xTq.ap().rearrange("(ko ki) s -> ki ko s", ki=128)
            )
            for m in range(KC):
                w_m = wmq.tile([128, KC, 128], F32R, tag="wmq")
                nc.sync.dma_start(w_m[:, :, :], wqT_r[:, :, m * 128:(m + 1) * 128])
                for n0 in range(0, SL, 512):
                    nw = min(512, SL - n0)
                    ps_full = psP.tile([128, 512], F32, tag="p1", name="p1ps")
                    ps = ps_full[:, :nw]
                    for k in range(KC):
                        nc.tensor.matmul(
                            ps, w_m[:, k, :], xTq_sb[:, k, n0:n0 + nw],
                            start=(k == 0), stop=(k == KC - 1),
                        )
                    nc.vector.tensor_copy(out=QT_sb[:, m, n0:n0 + nw], in_=ps)

        # ---------------- P1b: K and V projections (x^T resident)
        with tc.tile_pool(name="xp", bufs=1) as xp, \
             tc.tile_pool(name="wmp", bufs=1) as wmp, \
             tc.tile_pool(name="wnp", bufs=1) as wnp, \
             tc.tile_pool(name="p1w", bufs=2) as p1w:
            xT_sb = xp.tile([128, KC, S], F32R)
            xT_r = xT.ap().rearrange("(ko ki) s -> ki ko s", ki=128)
            for k in range(KC):
                eng = nc.sync if k % 2 == 0 else nc.gpsimd
                eng.dma_start(xT_sb[:, k, :], xT_r[:, k, :])

            def emit_k_block(m):
                w_m = wmp.tile([128, KC, 128], F32R, tag="wm", name="wm")
                nc.sync.dma_start(w_m[:, :, :], wkT_r[:, :, m * 128:(m + 1) * 128])
                for n0 in range(0, S, 512):
                    nw = min(512, S - n0)
                    ps_full = psP.tile([128, 512], F32, tag="p1", name="p1ps")
                    ps = ps_full[:, :nw]
                    for k in range(KC):
                        nc.tensor.matmul(
                            ps, w_m[:, k, :], xT_sb[:, k, n0:n0 + nw],
                            start=(k == 0), stop=(k == KC - 1),
                        )
                    nc.vector.tensor_copy(out=KT_sb[:, m, n0:n0 + nw], in_=ps)

            def emit_v_group(g):
                G = D // 4
                for c0 in range(0, G, 256):
                    hw_ = min(256, G - c0)
                    w_n = wnp.tile([128, KC, 256], F32R, tag="wn", name="wn")
                    nc.sync.dma_start(
                        w_n[:, :, :hw_],
                        wvT_r[:, :, g * G + c0:g * G + c0 + hw_],
                    )
                    for m in range(KB):
                        ps_full = psP.tile([128, 512], F32, tag="p1", name="p1ps")
                        ps = ps_full[:, :hw_]
                        for k in range(KC):
                            nc.tensor.matmul(
                                ps, xT_sb[:, k, m * 128:(m + 1) * 128],
                                w_n[:, k, :hw_],
                                start=(k == 0), stop=(k == KC - 1),
                            )
                        vrow = p1w.tile([128, 256], F32R, tag="vrow",
                                        name=f"vrow_{m}_{c0}")
                        nc.vector.tensor_copy(out=vrow[:, :hw_], in_=ps)
                        nc.gpsimd.dma_start(
                            vstages[g][m * 128:(m + 1) * 128, c0:c0 + hw_],
                            vrow[:, :hw_],
                        )

            # Interleave: Vg0 K0 Vg1 K1 Vg2 K2 Vg3 K3 K4..K7 so that head h
            # (needs KT block h//2 + V group h//4) unblocks progressively.
            emit_v_group(0)
            emit_k_block(0)
            emit_v_group(1)
            emit_k_block(1)
            emit_v_group(2)
            emit_k_block(2)
            emit_v_group(3)
            for m in range(3, KC):
                emit_k_block(m)

        # ---------------- P2: attention per head ----------------
        vstage_rs = [v[:].rearrange("(ko ki) d -> ki ko d", ki=128)
                     for v in vstages]
        kb_batches = _chunks3(KB)
        BMAX = SL * min(2, KB)

        for h in range(H):
            pi0 = (h % 2) * 64          # partition offset of this head's d rows
            po = h // 2                 # chunk index of this head's d rows
            QTh = QT_sb[pi0:pi0 + 64, po, :]

            HPG = max(1, H // 4)
            vg, vcol = h // HPG, (h % HPG) * DK
            vh0 = p2w.tile([128, KB2, DK], F32R, tag="vh", name="vh0")
            nc.gpsimd.dma_start(
                vh0[:, :, :], vstage_rs[vg][:, :KB2, vcol:vcol + DK],
            )
            vh1 = p2w.tile([128, KB - KB2, DK], F32R, tag="vh", name="vh1")
            nc.gpsimd.dma_start(
                vh1[:, :, :], vstage_rs[vg][:, KB2:, vcol:vcol + DK],
            )

            denoms = p2w.tile([128, QB, NA], F32, tag="denoms")
            psc = psC.tile([128, 512], F32, tag="ctx")

            # --- B pass: scores^T -> exp -> PV (context^T, unnormalized)
            for kbs in kb_batches:
                bps_full = psB.tile([128, BMAX], F32, tag="B", name="bps")
                bps = bps_full[:, :SL * len(kbs)]
                for i, kb in enumerate(kbs):
                    nc.tensor.matmul(
                        bps[:, SL * i:SL * (i + 1)],
                        KT_sb[pi0:pi0 + 64, po, kb * 128:(kb + 1) * 128],
                        QTh,
                        start=True, stop=True,
                    )
                pbt_full = probsp.tile([128, BMAX], F32R, tag="pbt", name="pbt")
                pbt = pbt_full[:, :SL * len(kbs)]
                nc.scalar.activation(out=pbt, in_=bps, func=AF.Exp, scale=scale)
                for i, kb in enumerate(kbs):
                    vht = vh0[:, kb, :] if kb < KB2 else vh1[:, kb - KB2, :]
                    nc.tensor.matmul(
                        psc[0:64, :SL],
                        vht,
                        pbt[:, SL * i:SL * (i + 1)],
                        start=(kb == 0), stop=(kb == KB - 1),
                    )

            # --- A pass: scores -> exp (+denom) -> normalize -> attn out
            for qb in range(QB):
                aps = psA.tile([128, ANW], F32, tag="A", name="aps")
                for w0 in range(0, ANW, 512):
                    ww = min(512, ANW - w0)
                    nc.tensor.matmul(
                        aps[:, w0:w0 + ww],
                        QT_sb[pi0:pi0 + 64, po, qb * 128:(qb + 1) * 128],
                        KT_sb[pi0:pi0 + 64, po, w0:w0 + ww],
                        start=True, stop=True,
                    )
                pa = pap.tile([128, ANW], F32, tag="pa", name="pa")
                nc.scalar.activation(
                    out=pa, in_=aps, func=AF.Exp, scale=scale,
                    accum_out=denoms[:, qb, 0:1],
                )
                rec = p2w.tile([128, QB], F32, tag="rec")
                nc.vector.reciprocal(rec[:, qb:qb + 1], denoms[:, qb, 0:1])
                HN = max(1, ANW // 1024)
                hw2 = ANW // HN
                for half in range(HN):
                    sl_ = slice(half * hw2, (half + 1) * hw2)
                    nc.vector.tensor_scalar(
                        pa[:, sl_], pa[:, sl_], rec[:, qb:qb + 1], None, ALU.mult
                    )
                    eng = nc.sync if (qb + half) % 2 == 0 else nc.gpsimd
                    eng.dma_start(
                        attn[h, qb * 128:(qb + 1) * 128, sl_], pa[:, sl_]
                    )

            # --- context^T scale by 1/denom (q on free dim) and place
            recq = p2w.tile([128, QB], F32, tag="rec2")
            nc.vector.reciprocal(recq[:, :], denoms[:, :, 0])
            # replicate recip 128-wide, transpose -> rows of per-q recip
            aps_t = psA.tile([128, ANW], F32, tag="A", name="apst")
            for c in range(QB):
                rec_rep = p2w.tile([128, 128], F32, tag="rec_rep")
                nc.vector.tensor_copy(
                    out=rec_rep[:, :],
                    in_=recq[:, c:c + 1].to_broadcast((128, 128)),
                )
                nc.tensor.transpose(
                    aps_t[:, 128 * c:128 * (c + 1)], rec_rep[:, :], ident,
                )
            Rrep = rrp.tile([128, SL], F32, tag="rrep", name="Rrep")
            nc.vector.tensor_copy(out=Rrep[0:64, :], in_=aps_t[0:64, :SL])
            if pi0 == 0:
                nc.vector.tensor_tensor(
                    ctxT_sb[0:64, po, :], psc[0:64, :SL], Rrep[0:64, :],
                    ALU.mult,
                )
            else:
                cstage = rrp.tile([128, SL], F32R, tag="cstage", name="cstage")
                nc.vector.tensor_tensor(
                    cstage[0:64, :], psc[0:64, :SL], Rrep[0:64, :], ALU.mult
                )
                nc.sync.dma_start(ctxT_sb[64:128, po, :], cstage[0:64, :])

        # ---------------- P3: out-proj + residual + LayerNorm ----------------
        with tc.tile_pool(name="p3w", bufs=3) as p3w, \
             tc.tile_pool(name="reps", bufs=1) as repsp:
            woT_sb = repsp.tile([128, KC, D], F32R)
            nc.gpsimd.dma_start(
                woT_sb[:, :, :], woT.ap().rearrange("(ko ki) d -> ki ko d", ki=128)
            )
            xr_sb = repsp.tile([128, QB, D], F32)
            nc.sync.dma_start(
                xr_sb[:, :, :], xrow.ap().rearrange("(qo qi) d -> qi qo d", qi=128)
            )

            def bcast_load(dr):
                t = repsp.tile([128, D], F32, tag=f"rep_{dr.name}",
                               name=f"rep_{dr.name}")
                src = dr[0:1, :]
                src_b = bass.AP(
                    tensor=src.tensor, offset=src.offset, ap=[[0, 128]] + src.ap[1:]
                )
                nc.gpsimd.dma_start(t[:, :], src_b)
                return t

            bo_rep = bcast_load(bo)
            g_rep = bcast_load(gam)
            b_rep = bcast_load(bet)

            fmax = int(np.gcd(512, D))
            nsub = D // fmax
            for m in range(QB):
                out_t = p3w.tile([128, D], F32, tag="out")
                for n0 in range(0, D, 512):
                    nw = min(512, D - n0)
                    ps_full = psP.tile([128, 512], F32, tag="p1", name="ops")
                    ps = ps_full[:, :nw]
                    for k in range(KC):
                        nc.tensor.matmul(
                            ps, ctxT_sb[:, k, m * 128:(m + 1) * 128],
                            woT_sb[:, k, n0:n0 + nw],
                            start=(k == 0), stop=(k == KC - 1),
                        )
                    # out = psum + b_o  (fused copy+bias)
                    nc.vector.tensor_tensor(
                        out_t[:, n0:n0 + nw], ps, bo_rep[:, n0:n0 + nw], ALU.add
                    )
                # residual
                nc.vector.tensor_tensor(
                    out_t[:, :], out_t[:, :], xr_sb[:, m, :], ALU.add
                )
                # LayerNorm
                stats = p3w.tile([128, nsub, 6], F32, tag="st")
                for sg in range(nsub):
                    nc.vector.bn_stats(
                        out=stats[:, sg, :], in_=out_t[:, sg * fmax:(sg + 1) * fmax]
                    )
                mv = p3w.tile([128, 2], F32, tag="mv")
                nc.vector.bn_aggr(out=mv[:, :], in_=stats[:, :, :])
                rstd = p3w.tile([128, 1], F32, tag="rstd")
                nc.scalar.activation(
                    out=rstd[:, :], in_=mv[:, 1:2], func=AF.Sqrt, bias=eps_sb[:, :]
                )
                nc.vector.reciprocal(rstd[:, :], rstd[:, :])
                nc.vector.tensor_scalar(
                    out_t[:, :], out_t[:, :], mv[:, 0:1], rstd[:, :],
                    ALU.subtract, ALU.mult,
                )
                nc.vector.tensor_tensor(out_t[:, :], out_t[:, :], g_rep[:, :], ALU.mult)
                nc.vector.tensor_tensor(out_t[:, :], out_t[:, :], b_rep[:, :], ALU.add)
                nc.gpsimd.dma_start(y[m * 128:(m + 1) * 128, :], out_t[:, :])

    nc.finalize()
    return nc


_NC_CACHE = {}


def _get_nc(key=(D_MODEL, SEQ, N_HEADS)):
    if key not in _NC_CACHE:
        D, S, H = key
        _NC_CACHE[key] = build_nc(D=D, S=S, H=H)
    return _NC_CACHE[key]


def kernel(x, W_q, W_k, W_v, W_o, b_o, ln_gamma, ln_beta):
    x = np.asarray(x, dtype=np.float32)
    B, S, D = x.shape
    H = N_HEADS
    SL = S // 4
    nc = _get_nc((D, S, H))

    wqT = np.ascontiguousarray(np.asarray(W_q, np.float32).T)
    wkT = np.ascontiguousarray(np.asarray(W_k, np.float32).T)
    wvT = np.ascontiguousarray(np.asarray(W_v, np.float32).T)
    woT = np.ascontiguousarray(np.asarray(W_o, np.float32).T)
    bo = np.asarray(b_o, np.float32).reshape(1, D)
    gam = np.asarray(ln_gamma, np.float32).reshape(1, D)
    bet = np.asarray(ln_beta, np.float32).reshape(1, D)

    in_maps = []
    for c in range(N_CORES):
        b, t = c // 4, c % 4
        xT = np.ascontiguousarray(x[b].T)
        in_maps.append({
            "xT": xT,
            "xTq": np.ascontiguousarray(xT[:, SL * t:SL * (t + 1)]),
            "xrow": np.ascontiguousarray(x[b, SL * t:SL * (t + 1), :]),
            "wqT": wqT, "wkT": wkT, "wvT": wvT, "woT": woT,
            "bo": bo, "gam": gam, "bet": bet,
        })

    res = run_bass_kernel_spmd(nc, in_maps, core_ids=list(range(N_CORES)))

    y = np.empty((B, S, D), np.float32)
    attn = np.empty((B, H, S, S), np.float32)
    for c in range(N_CORES):
        b, t = c // 4, c % 4
        y[b, SL * t:SL * (t + 1), :] = res.results[c]["y"]
        attn[b, :, SL * t:SL * (t + 1), :] = res.results[c]["attn"]
    return y, attn


# revision 34
# speedup vs baseline: 1.6659x; 1.0820x over previous
"""Trainium2 Bass kernel for nn_MultiHeadAttention (fused MHA + residual + LayerNorm).

Sharding: 8 cores = 2 batches x 4 query-slices of 512 tokens. Each core:
  - projects Q for its 512-token slice (transposed layout QT [D, SL])
  - projects K (transposed, full seq) and V (natural, full seq) for its batch
    (replicated across the 4 cores of the batch -> zero collectives)
  - computes scores^T per head (k-major) -> exp -> PV matmul -> context^T
  - computes scores per head (q-major) -> exp (accum_out = softmax denom)
    -> normalize -> writes attn probs output
  - output projection + bias + residual + LayerNorm -> y slice

Matmul inputs use float32r (fast fp32 streaming mode, 1 cycle/row for N>=256).
Pool stack is ordered so attention-phase tiles sit below the projection-phase
x^T buffer: the scheduler can overlap projections with early heads' attention.

kernel(**inputs) takes FULL inputs, shards on host, runs 8-core SPMD,
reassembles FULL outputs (y, attn).
"""
from contextlib import ExitStack

import numpy as np

import concourse.bass as bass
import concourse.tile as tile
from concourse import bacc, mybir
from concourse.masks import make_identity
from concourse.bass_utils import run_bass_kernel_spmd

F32 = mybir.dt.float32
F32R = mybir.dt.float32r
AF = mybir.ActivationFunctionType
ALU = mybir.AluOpType

# Full-size problem constants (hardcoded per harness contract)
D_MODEL = 1024
SEQ = 2048
N_HEADS = 16
D_K = 64
LN_EPS = 1e-5
N_CORES = 8


def _chunks3(n, c=2):
    out, i = [], 0
    while i < n:
        out.append(list(range(i, min(i + c, n))))
        i += c
    return out


def build_nc(D=D_MODEL, S=SEQ, SL=None, H=N_HEADS, eps=LN_EPS, trace_sim=False):
    """Build the per-core SPMD program. SL = this core's query-slice length."""
    if SL is None:
        SL = S // 4
    DK = D // H
    assert DK == 64, "head packing assumes d_k == 64"
    KC = D // 128   # contraction chunks over d_model
    KB = S // 128   # key blocks
    QB = SL // 128  # query blocks in slice
    KB2 = (KB + 1) // 2
    ANW = min(2048, S)    # A-pass psum region width (4 banks)
    NA = S // ANW         # A-pass activates per query block (1)
    scale = 1.0 / float(np.sqrt(DK))

    nc = bacc.Bacc("TRN2", debug=False)

    xT = nc.dram_tensor("xT", [D, S], F32R, kind="ExternalInput")
    xTq = nc.dram_tensor("xTq", [D, SL], F32R, kind="ExternalInput")
    xrow = nc.dram_tensor("xrow", [SL, D], F32, kind="ExternalInput")
    wqT = nc.dram_tensor("wqT", [D, D], F32R, kind="ExternalInput")
    wkT = nc.dram_tensor("wkT", [D, D], F32R, kind="ExternalInput")
    wvT = nc.dram_tensor("wvT", [D, D], F32R, kind="ExternalInput")
    woT = nc.dram_tensor("woT", [D, D], F32R, kind="ExternalInput")
    bo = nc.dram_tensor("bo", [1, D], F32, kind="ExternalInput")
    gam = nc.dram_tensor("gam", [1, D], F32, kind="ExternalInput")
    bet = nc.dram_tensor("bet", [1, D], F32, kind="ExternalInput")
    attn = nc.dram_tensor("attn", [H, SL, S], F32, kind="ExternalOutput")
    y = nc.dram_tensor("y", [SL, D], F32, kind="ExternalOutput")

    with tile.TileContext(nc, trace_sim=trace_sim) as tc, ExitStack() as ctx:
        # ---- program-lifetime pools, attention tiles below x^T on the stack
        pers = ctx.enter_context(tc.tile_pool(name="pers", bufs=1))
        dram = ctx.enter_context(tc.tile_pool(name="dram", bufs=1, space="DRAM"))
        probsp = ctx.enter_context(tc.tile_pool(name="probs", bufs=2))
        pap = ctx.enter_context(tc.tile_pool(name="pap", bufs=2))
        p2w = ctx.enter_context(tc.tile_pool(name="p2w", bufs=2))
        late = ctx.enter_context(tc.tile_pool(name="late", bufs=1))
        rrp = ctx.enter_context(tc.tile_pool(name="rrp", bufs=1))
        psP = ctx.enter_context(tc.tile_pool(name="psP", bufs=1, space="PSUM"))
        psA = ctx.enter_context(tc.tile_pool(name="psA", bufs=1, space="PSUM"))
        psB = ctx.enter_context(tc.tile_pool(name="psB", bufs=1, space="PSUM"))
        psC = ctx.enter_context(tc.tile_pool(name="psC", bufs=1, space="PSUM"))

        ident = pers.tile([128, 128], F32)
        make_identity(nc, ident)
        eps_sb = pers.tile([128, 1], F32)
        nc.vector.memset(eps_sb, eps)

        QT_sb = pers.tile([128, KC, SL], F32R)     # Q^T, d on partitions
        KT_sb = pers.tile([128, KC, S], F32R)      # K^T, d on partitions
        ctxT_sb = late.tile([128, KC, SL], F32R)   # context^T, dv on partitions
        vstages = [dram.tile([S, D // 4], F32R, tag=f"vs{g}", name=f"vs{g}")
                   for g in range(4)]

        wqT_r = wqT.ap().rearrange("(ko ki) d -> ki ko d", ki=128)
        wkT_r = wkT.ap().rearrange("(ko ki) d -> ki ko d", ki=128)
        wvT_r = wvT.ap().rearrange("(ko ki) d -> ki ko d", ki=128)

        # ---------------- P1a: Q projection (small transient x^T slice)
        with tc.tile_pool(name="xpq", bufs=1) as xpq, \
             tc.tile_pool(name="wmq", bufs=2) as wmq:
            xTq_sb = xpq.tile([128, KC, SL], F32R)
            nc.gpsimd.dma_start(
                xTq_sb[:, :, :], xTq.ap().rearrange("(ko ki) s -> ki ko s", ki=128)
            )
            for m in range(KC):
                w_m = wmq.tile([128, KC, 128], F32R, tag="wmq")
                nc.sync.dma_start(w_m[:, :, :], wqT_r[:, :, m * 128:(m + 1) * 128])
                for n0 in range(0, SL, 512):
                    nw = min(512, SL - n0)
                    ps_full = psP.tile([128, 512], F32, tag="p1", name="p1ps")
                    ps = ps_full[:, :nw]
                    for k in range(KC):
                        nc.tensor.matmul(
                            ps, w_m[:, k, :], xTq_sb[:, k, n0:n0 + nw],
                            start=(k == 0), stop=(k == KC - 1),
                        )
                    nc.vector.tensor_copy(out=QT_sb[:, m, n0:n0 + nw], in_=ps)

        # ---------------- P1b: K and V projections (x^T resident)
        with tc.tile_pool(name="xp", bufs=1) as xp, \
             tc.tile_pool(name="wmp", bufs=1) as wmp, \
             tc.tile_pool(name="wnp", bufs=1) as wnp, \
             tc.tile_pool(name="p1w", bufs=2) as p1w:
            xT_sb = xp.tile([128, KC, S], F32R)
            xT_r = xT.ap().rearrange("(ko ki) s -> ki ko s", ki=128)
            for k in range(KC):
                eng = nc.sync if k % 2 == 0 else nc.gpsimd
                eng.dma_start(xT_sb[:, k, :], xT_r[:, k, :])

            def emit_k_block(m):
                w_m = wmp.tile([128, KC, 128], F32R, tag="wm", name="wm")
                nc.sync.dma_start(w_m[:, :, :], wkT_r[:, :, m * 128:(m + 1) * 128])
                for n0 in range(0, S, 512):
                    nw = min(512, S - n0)
                    ps_full = psP.tile([128, 512], F32, tag="p1", name="p1ps")
                    ps = ps_full[:, :nw]
                    for k in range(KC):
                        nc.tensor.matmul(
                            ps, w_m[:, k, :], xT_sb[:, k, n0:n0 + nw],
                            start=(k == 0), stop=(k == KC - 1),
                        )
                    nc.vector.tensor_copy(out=KT_sb[:, m, n0:n0 + nw], in_=ps)

            def emit_v_group(g):
                G = D // 4
                for c0 in range(0, G, 256):
                    hw_ = min(256, G - c0)
                    w_n = wnp.tile([128, KC, 256], F32R, tag="wn", name="wn")
                    nc.sync.dma_start(
                        w_n[:, :, :hw_],
                        wvT_r[:, :, g * G + c0:g * G + c0 + hw_],
                    )
                    for m in range(KB):
                        ps_full = psP.tile([128, 512], F32, tag="p1", name="p1ps")
                        ps = ps_full[:, :hw_]
                        for k in range(KC):
                            nc.tensor.matmul(
                                ps, xT_sb[:, k, m * 128:(m + 1) * 128],
                                w_n[:, k, :hw_],
                                start=(k == 0), stop=(k == KC - 1),
                            )
                        vrow = p1w.tile([128, 256], F32R, tag="vrow",
                                        name=f"vrow_{m}_{c0}")
                        nc.vector.tensor_copy(out=vrow[:, :hw_], in_=ps)
                        nc.gpsimd.dma_start(
                            vstages[g][m * 128:(m + 1) * 128, c0:c0 + hw_],
                            vrow[:, :hw_],
                        )

            # Interleave: Vg0 K0 Vg1 K1 Vg2 K2 Vg3 K3 K4..K7 so that head h
            # (needs KT block h//2 + V group h//4) unblocks progressively.
            emit_v_group(0)
            emit_k_block(0)
            emit_v_group(1)
            emit_k_block(1)
            emit_v_group(2)
            emit_k_block(2)
            emit_v_group(3)
            for m in range(3, KC):
                emit_k_block(m)

        # ---------------- P2: attention per head ----------------
        vstage_rs = [v[:].rearrange("(ko ki) d -> ki ko d", ki=128)
                     for v in vstages]
        kb_batches = _chunks3(KB)
        BMAX = SL * min(2, KB)

        for h in range(H):
            pi0 = (h % 2) * 64          # partition offset of this head's d rows
            po = h // 2                 # chunk index of this head's d rows
            QTh = QT_sb[pi0:pi0 + 64, po, :]

            HPG = max(1, H // 4)
            vg, vcol = h // HPG, (h % HPG) * DK
            vh0 = p2w.tile([128, KB2, DK], F32R, tag="vh", name="vh0")
            nc.gpsimd.dma_start(
                vh0[:, :, :], vstage_rs[vg][:, :KB2, vcol:vcol + DK],
            )
            vh1 = p2w.tile([128, KB - KB2, DK], F32R, tag="vh", name="vh1")
            nc.gpsimd.dma_start(
                vh1[:, :, :], vstage_rs[vg][:, KB2:, vcol:vcol + DK],
            )

            denoms = p2w.tile([128, QB, NA], F32, tag="denoms")
            psc = psC.tile([128, 512], F32, tag="ctx")

            # --- B pass: scores^T -> exp -> PV (context^T, unnormalized)
            for kbs in kb_batches:
                bps_full = psB.tile([128, BMAX], F32, tag="B", name="bps")
                bps = bps_full[:, :SL * len(kbs)]
                for i, kb in enumerate(kbs):
                    nc.tensor.matmul(
                        bps[:, SL * i:SL * (i + 1)],
                        KT_sb[pi0:pi0 + 64, po, kb * 128:(kb + 1) * 128],
                        QTh,
                        start=True, stop=True,
                    )
                pbt_full = probsp.tile([128, BMAX], F32R, tag="pbt", name="pbt")
                pbt = pbt_full[:, :SL * len(kbs)]
                nc.scalar.activation(out=pbt, in_=bps, func=AF.Exp, scale=scale)
                for i, kb in enumerate(kbs):
                    vht = vh0[:, kb, :] if kb < KB2 else vh1[:, kb - KB2, :]
                    nc.tensor.matmul(
                        psc[0:64, :SL],
                        vht,
                        pbt[:, SL * i:SL * (i + 1)],
                        start=(kb == 0), stop=(kb == KB - 1),
                    )

            # --- A pass: scores -> exp (+denom) -> normalize -> attn out
            for qb in range(QB):
                aps = psA.tile([128, ANW], F32, tag="A", name="aps")
                for w0 in range(0, ANW, 512):
                    ww = min(512, ANW - w0)
                    nc.tensor.matmul(
                        aps[:, w0:w0 + ww],
                        QT_sb[pi0:pi0 + 64, po, qb * 128:(qb + 1) * 128],
                        KT_sb[pi0:pi0 + 64, po, w0:w0 + ww],
                        start=True, stop=True,
                    )
                pa = pap.tile([128, ANW], F32, tag="pa", name="pa")
                nc.scalar.activation(
                    out=pa, in_=aps, func=AF.Exp, scale=scale,
                    accum_out=denoms[:, qb, 0:1],
                )
                rec = p2w.tile([128, QB], F32, tag="rec")
                nc.vector.reciprocal(rec[:, qb:qb + 1], denoms[:, qb, 0:1])
                HN = max(1, ANW // 1024)
                hw2 = ANW // HN
                for half in range(HN):
                    sl_ = slice(half * hw2, (half + 1) * hw2)
                    nc.vector.tensor_scalar(
                        pa[:, sl_], pa[:, sl_], rec[:, qb:qb + 1], None, ALU.mult
                    )
                    eng = nc.sync if (qb + half) % 2 == 0 else nc.gpsimd
                    eng.dma_start(
                        attn[h, qb * 128:(qb + 1) * 128, sl_], pa[:, sl_]
                    )

            # --- context^T scale by 1/denom (q on free dim) and place
            recq = p2w.tile([128, QB], F32, tag="rec2")
            nc.vector.reciprocal(recq[:, :], denoms[:, :, 0])
            # replicate recip 128-wide, transpose -> rows of per-q recip
            aps_t = psA.tile([128, ANW], F32, tag="A", name="apst")
            for c in range(QB):
                rec_rep = p2w.tile([128, 128], F32, tag="rec_rep")
                nc.vector.tensor_copy(
                    out=rec_rep[:, :],
                    in_=recq[:, c:c + 1].to_broadcast((128, 128)),
                )
                nc.tensor.transpose(
                    aps_t[:, 128 * c:128 * (c + 1)], rec_rep[:, :], ident,
                )
            Rrep = rrp.tile([128, SL], F32, tag="rrep", name="Rrep")
            nc.vector.tensor_copy(out=Rrep[0:64, :], in_=aps_t[0:64, :SL])
            if pi0 == 0:
                nc.vector.tensor_tensor(
                    ctxT_sb[0:64, po, :], psc[0:64, :SL], Rrep[0:64, :],
                    ALU.mult,
                )
            else:
                cstage = rrp.tile([128, SL], F32R, tag="cstage", name="cstage")
                nc.vector.tensor_tensor(
                    cstage[0:64, :], psc[0:64, :SL], Rrep[0:64, :], ALU.mult
                )
                nc.sync.dma_start(ctxT_sb[64:128, po, :], cstage[0:64, :])

        # ---------------- P3: out-proj + residual + LayerNorm ----------------
        with tc.tile_pool(name="p3w", bufs=3) as p3w, \
             tc.tile_pool(name="reps", bufs=1) as repsp:
            woT_sb = repsp.tile([128, KC, D], F32R)
            nc.gpsimd.dma_start(
                woT_sb[:, :, :], woT.ap().rearrange("(ko ki) d -> ki ko d", ki=128)
            )
            xr_sb = repsp.tile([128, QB, D], F32)
            nc.sync.dma_start(
                xr_sb[:, :, :], xrow.ap().rearrange("(qo qi) d -> qi qo d", qi=128)
            )

            def bcast_load(dr):
                t = repsp.tile([128, D], F32, tag=f"rep_{dr.name}",
                               name=f"rep_{dr.name}")
                src = dr[0:1, :]
                src_b = bass.AP(
                    tensor=src.tensor, offset=src.offset, ap=[[0, 128]] + src.ap[1:]
                )
                nc.gpsimd.dma_start(t[:, :], src_b)
                return t

            bo_rep = bcast_load(bo)
            g_rep = bcast_load(gam)
            b_rep = bcast_load(bet)

            fmax = int(np.gcd(512, D))
            nsub = D // fmax
            for m in range(QB):
                out_t = p3w.tile([128, D], F32, tag="out")
                # alternate between the (idle) attention psum regions so
                # m-block matmuls overlap the previous block's LayerNorm
                if m % 2 == 0:
                    preg = psA.tile([128, ANW], F32, tag="A", name="opsA")
                else:
                    preg = psB.tile([128, BMAX], F32, tag="B", name="opsB")
                for n0 in range(0, D, 512):
                    nw = min(512, D - n0)
                    ps = preg[:, n0 % 1024:n0 % 1024 + nw]
                    for k in range(KC):
                        nc.tensor.matmul(
                            ps, ctxT_sb[:, k, m * 128:(m + 1) * 128],
                            woT_sb[:, k, n0:n0 + nw],
                            start=(k == 0), stop=(k == KC - 1),
                        )
                    # out = psum + b_o  (fused copy+bias)
                    nc.vector.tensor_tensor(
                        out_t[:, n0:n0 + nw], ps, bo_rep[:, n0:n0 + nw], ALU.add
                    )
                # residual
                nc.vector.tensor_tensor(
                    out_t[:, :], out_t[:, :], xr_sb[:, m, :], ALU.add
                )
                # LayerNorm
                stats = p3w.tile([128, nsub, 6], F32, tag="st")
                for sg in range(nsub):
                    nc.vector.bn_stats(
                        out=stats[:, sg, :], in_=out_t[:, sg * fmax:(sg + 1) * fmax]
                    )
                mv = p3w.tile([128, 2], F32, tag="mv")
                nc.vector.bn_aggr(out=mv[:, :], in_=stats[:, :, :])
                rstd = p3w.tile([128, 1], F32, tag="rstd")
                nc.scalar.activation(
                    out=rstd[:, :], in_=mv[:, 1:2], func=AF.Sqrt, bias=eps_sb[:, :]
                )
                nc.vector.reciprocal(rstd[:, :], rstd[:, :])
                nc.vector.tensor_scalar(
                    out_t[:, :], out_t[:, :], mv[:, 0:1], rstd[:, :],
                    ALU.subtract, ALU.mult,
                )
                nc.vector.tensor_tensor(out_t[:, :], out_t[:, :], g_rep[:, :], ALU.mult)
                nc.vector.tensor_tensor(out_t[:, :], out_t[:, :], b_rep[:, :], ALU.add)
                nc.gpsimd.dma_start(y[m * 128:(m + 1) * 128, :], out_t[:, :])

    nc.finalize()
    return nc


_NC_CACHE = {}


def _get_nc(key=(D_MODEL, SEQ, N_HEADS)):
    if key not in _NC_CACHE:
        D, S, H = key
        _NC_CACHE[key] = build_nc(D=D, S=S, H=H)
    return _NC_CACHE[key]


def kernel(x, W_q, W_k, W_v, W_o, b_o, ln_gamma, ln_beta):
    x = np.asarray(x, dtype=np.float32)
    B, S, D = x.shape
    H = N_HEADS
    SL = S // 4
    nc = _get_nc((D, S, H))

    wqT = np.ascontiguousarray(np.asarray(W_q, np.float32).T)
    wkT = np.ascontiguousarray(np.asarray(W_k, np.float32).T)
    wvT = np.ascontiguousarray(np.asarray(W_v, np.float32).T)
    woT = np.ascontiguousarray(np.asarray(W_o, np.float32).T)
    bo = np.asarray(b_o, np.float32).reshape(1, D)
    gam = np.asarray(ln_gamma, np.float32).reshape(1, D)
    bet = np.asarray(ln_beta, np.float32).reshape(1, D)

    in_maps = []
    for c in range(N_CORES):
        b, t = c // 4, c % 4
        xT = np.ascontiguousarray(x[b].T)
        in_maps.append({
            "xT": xT,
            "xTq": np.ascontiguousarray(xT[:, SL * t:SL * (t + 1)]),
            "xrow": np.ascontiguousarray(x[b, SL * t:SL * (t + 1), :]),
            "wqT": wqT, "wkT": wkT, "wvT": wvT, "woT": woT,
            "bo": bo, "gam": gam, "bet": bet,
        })

    res = run_bass_kernel_spmd(nc, in_maps, core_ids=list(range(N_CORES)))

    y = np.empty((B, S, D), np.float32)
    attn = np.empty((B, H, S, S), np.float32)
    for c in range(N_CORES):
        b, t = c // 4, c % 4
        y[b, SL * t:SL * (t + 1), :] = res.results[c]["y"]
        attn[b, :, SL * t:SL * (t + 1), :] = res.results[c]["attn"]
    return y, attn


# revision 35
# speedup vs baseline: 2.0052x; 1.2036x over previous
"""Trainium2 Bass kernel for nn_MultiHeadAttention (fused MHA + residual + LayerNorm).

Sharding: 8 cores = 2 batches x 4 query-slices of 512 tokens. Each core:
  - projects Q for its 512-token slice (transposed layout QT [D, SL])
  - projects K (transposed, full seq) and V (natural, full seq) for its batch
    (replicated across the 4 cores of the batch -> zero collectives)
  - computes scores^T per head (k-major) -> exp -> PV matmul -> context^T
  - computes scores per head (q-major) -> exp (accum_out = softmax denom)
    -> normalize -> writes attn probs output
  - output projection + bias + residual + LayerNorm -> y slice

Matmul inputs use float32r (fast fp32 streaming mode, 1 cycle/row for N>=256).
Pool stack is ordered so attention-phase tiles sit below the projection-phase
x^T buffer: the scheduler can overlap projections with early heads' attention.

kernel(**inputs) takes FULL inputs, shards on host, runs 8-core SPMD,
reassembles FULL outputs (y, attn).
"""
from contextlib import ExitStack

import numpy as np

import concourse.bass as bass
import concourse.tile as tile
from concourse import bacc, mybir
from concourse.masks import make_identity
from concourse.bass_utils import run_bass_kernel_spmd

F32 = mybir.dt.float32
F32R = mybir.dt.float32r
AF = mybir.ActivationFunctionType
ALU = mybir.AluOpType

# Full-size problem constants (hardcoded per harness contract)
D_MODEL = 1024
SEQ = 2048
N_HEADS = 16
D_K = 64
LN_EPS = 1e-5
N_CORES = 8


def _chunks3(n, c=2):
    out, i = [], 0
    while i < n:
        out.append(list(range(i, min(i + c, n))))
        i += c
    return out


def build_nc(D=D_MODEL, S=SEQ, SL=None, H=N_HEADS, eps=LN_EPS, trace_sim=False):
    """Build the per-core SPMD program. SL = this core's query-slice length."""
    if SL is None:
        SL = S // 4
    DK = D // H
    assert DK == 64, "head packing assumes d_k == 64"
    KC = D // 128   # contraction chunks over d_model
    KB = S // 128   # key blocks
    QB = SL // 128  # query blocks in slice
    KB2 = (KB + 1) // 2
    ANW = min(2048, S)    # A-pass psum region width (4 banks)
    NA = S // ANW         # A-pass activates per query block (1)
    scale = 1.0 / float(np.sqrt(DK))

    nc = bacc.Bacc("TRN2", debug=False)

    xT = nc.dram_tensor("xT", [D, S], F32R, kind="ExternalInput")
    xTq = nc.dram_tensor("xTq", [D, SL], F32R, kind="ExternalInput")
    xrow = nc.dram_tensor("xrow", [SL, D], F32, kind="ExternalInput")
    wqT = nc.dram_tensor("wqT", [D, D], F32R, kind="ExternalInput")
    wkT = nc.dram_tensor("wkT", [D, D], F32R, kind="ExternalInput")
    wvT = nc.dram_tensor("wvT", [D, D], F32R, kind="ExternalInput")
    woT = nc.dram_tensor("woT", [D, D], F32R, kind="ExternalInput")
    bo = nc.dram_tensor("bo", [1, D], F32, kind="ExternalInput")
    gam = nc.dram_tensor("gam", [1, D], F32, kind="ExternalInput")
    bet = nc.dram_tensor("bet", [1, D], F32, kind="ExternalInput")
    attn = nc.dram_tensor("attn", [H, SL, S], F32, kind="ExternalOutput")
    y = nc.dram_tensor("y", [SL, D], F32, kind="ExternalOutput")

    with tile.TileContext(nc, trace_sim=trace_sim) as tc, ExitStack() as ctx:
        # ---- program-lifetime pools, attention tiles below x^T on the stack
        pers = ctx.enter_context(tc.tile_pool(name="pers", bufs=1))
        dram = ctx.enter_context(tc.tile_pool(name="dram", bufs=1, space="DRAM"))
        probsp = ctx.enter_context(tc.tile_pool(name="probs", bufs=2))
        pap = ctx.enter_context(tc.tile_pool(name="pap", bufs=2))
        p2w = ctx.enter_context(tc.tile_pool(name="p2w", bufs=2))
        late = ctx.enter_context(tc.tile_pool(name="late", bufs=1))
        rrp = ctx.enter_context(tc.tile_pool(name="rrp", bufs=1))
        psP = ctx.enter_context(tc.tile_pool(name="psP", bufs=1, space="PSUM"))
        psA = ctx.enter_context(tc.tile_pool(name="psA", bufs=1, space="PSUM"))
        psB = ctx.enter_context(tc.tile_pool(name="psB", bufs=1, space="PSUM"))
        psC = ctx.enter_context(tc.tile_pool(name="psC", bufs=1, space="PSUM"))

        ident = pers.tile([128, 128], F32)
        make_identity(nc, ident)
        eps_sb = pers.tile([128, 1], F32)
        nc.vector.memset(eps_sb, eps)

        QT_sb = pers.tile([128, KC, SL], F32R)     # Q^T, d on partitions
        KT_sb = pers.tile([128, KC, S], F32R)      # K^T, d on partitions
        ctxT_sb = late.tile([128, KC, SL], F32R)   # context^T, dv on partitions
        vstages = [dram.tile([S, D // 4], F32R, tag=f"vs{g}", name=f"vs{g}")
                   for g in range(4)]

        wqT_r = wqT.ap().rearrange("(ko ki) d -> ki ko d", ki=128)
        wkT_r = wkT.ap().rearrange("(ko ki) d -> ki ko d", ki=128)
        wvT_r = wvT.ap().rearrange("(ko ki) d -> ki ko d", ki=128)

        # ---------------- P1a: Q projection (small transient x^T slice)
        with tc.tile_pool(name="xpq", bufs=1) as xpq, \
             tc.tile_pool(name="wmq", bufs=2) as wmq:
            xTq_sb = xpq.tile([128, KC, SL], F32R)
            nc.gpsimd.dma_start(
                xTq_sb[:, :, :], xTq.ap().rearrange("(ko ki) s -> ki ko s", ki=128)
            )
            for m in range(KC):
                w_m = wmq.tile([128, KC, 128], F32R, tag="wmq")
                nc.sync.dma_start(w_m[:, :, :], wqT_r[:, :, m * 128:(m + 1) * 128])
                for n0 in range(0, SL, 512):
                    nw = min(512, SL - n0)
                    ps_full = psP.tile([128, 512], F32, tag="p1", name="p1ps")
                    ps = ps_full[:, :nw]
                    for k in range(KC):
                        nc.tensor.matmul(
                            ps, w_m[:, k, :], xTq_sb[:, k, n0:n0 + nw],
                            start=(k == 0), stop=(k == KC - 1),
                        )
                    nc.vector.tensor_copy(out=QT_sb[:, m, n0:n0 + nw], in_=ps)

        # ---------------- P1b: K and V projections (x^T resident)
        with tc.tile_pool(name="xp", bufs=1) as xp, \
             tc.tile_pool(name="wmp", bufs=1) as wmp, \
             tc.tile_pool(name="wnp", bufs=1) as wnp, \
             tc.tile_pool(name="p1w", bufs=2) as p1w:
            xT_sb = xp.tile([128, KC, S], F32R)
            xT_r = xT.ap().rearrange("(ko ki) s -> ki ko s", ki=128)
            for k in range(KC):
                eng = nc.sync if k % 2 == 0 else nc.gpsimd
                eng.dma_start(xT_sb[:, k, :], xT_r[:, k, :])

            def emit_k_block(m):
                w_m = wmp.tile([128, KC, 128], F32R, tag="wm", name="wm")
                nc.sync.dma_start(w_m[:, :, :], wkT_r[:, :, m * 128:(m + 1) * 128])
                for n0 in range(0, S, 512):
                    nw = min(512, S - n0)
                    ps_full = psP.tile([128, 512], F32, tag="p1", name="p1ps")
                    ps = ps_full[:, :nw]
                    for k in range(KC):
                        nc.tensor.matmul(
                            ps, w_m[:, k, :], xT_sb[:, k, n0:n0 + nw],
                            start=(k == 0), stop=(k == KC - 1),
                        )
                    nc.vector.tensor_copy(out=KT_sb[:, m, n0:n0 + nw], in_=ps)

            def emit_v_group(g):
                G = D // 4
                for c0 in range(0, G, 256):
                    hw_ = min(256, G - c0)
                    w_n = wnp.tile([128, KC, 256], F32R, tag="wn", name="wn")
                    nc.sync.dma_start(
                        w_n[:, :, :hw_],
                        wvT_r[:, :, g * G + c0:g * G + c0 + hw_],
                    )
                    for m in range(KB):
                        ps_full = psP.tile([128, 512], F32, tag="p1", name="p1ps")
                        ps = ps_full[:, :hw_]
                        for k in range(KC):
                            nc.tensor.matmul(
                                ps, xT_sb[:, k, m * 128:(m + 1) * 128],
                                w_n[:, k, :hw_],
                                start=(k == 0), stop=(k == KC - 1),
                            )
                        vrow = p1w.tile([128, 256], F32R, tag="vrow",
                                        name=f"vrow_{m}_{c0}")
                        nc.vector.tensor_copy(out=vrow[:, :hw_], in_=ps)
                        nc.gpsimd.dma_start(
                            vstages[g][m * 128:(m + 1) * 128, c0:c0 + hw_],
                            vrow[:, :hw_],
                        )

            # Interleave: Vg0 K0 Vg1 K1 Vg2 K2 Vg3 K3..  so that head h
            # (needs KT block h//2 + V group h//4) unblocks progressively.
            emitted_k = 0
            for g in range(4):
                emit_v_group(g)
                if emitted_k < KC:
                    emit_k_block(emitted_k)
                    emitted_k += 1
            for m in range(emitted_k, KC):
                emit_k_block(m)

        # ---------------- P2: attention per head ----------------
        vstage_rs = [v[:].rearrange("(ko ki) d -> ki ko d", ki=128)
                     for v in vstages]
        kb_batches = _chunks3(KB)
        BMAX = SL * min(2, KB)

        for h in range(H):
            pi0 = (h % 2) * 64          # partition offset of this head's d rows
            po = h // 2                 # chunk index of this head's d rows
            QTh = QT_sb[pi0:pi0 + 64, po, :]

            HPG = max(1, H // 4)
            vg, vcol = h // HPG, (h % HPG) * DK
            vh0 = p2w.tile([128, KB2, DK], F32R, tag="vh", name="vh0")
            nc.gpsimd.dma_start(
                vh0[:, :, :], vstage_rs[vg][:, :KB2, vcol:vcol + DK],
            )
            vh1 = p2w.tile([128, KB - KB2, DK], F32R, tag="vh", name="vh1")
            nc.gpsimd.dma_start(
                vh1[:, :, :], vstage_rs[vg][:, KB2:, vcol:vcol + DK],
            )

            denoms = p2w.tile([128, QB, NA], F32, tag="denoms")
            psc = psC.tile([128, 512], F32, tag="ctx")

            # --- B pass: scores^T -> exp -> PV (context^T, unnormalized)
            for kbs in kb_batches:
                bps_full = psB.tile([128, BMAX], F32, tag="B", name="bps")
                bps = bps_full[:, :SL * len(kbs)]
                for i, kb in enumerate(kbs):
                    nc.tensor.matmul(
                        bps[:, SL * i:SL * (i + 1)],
                        KT_sb[pi0:pi0 + 64, po, kb * 128:(kb + 1) * 128],
                        QTh,
                        start=True, stop=True,
                    )
                pbt_full = probsp.tile([128, BMAX], F32R, tag="pbt", name="pbt")
                pbt = pbt_full[:, :SL * len(kbs)]
                nc.scalar.activation(out=pbt, in_=bps, func=AF.Exp, scale=scale)
                for i, kb in enumerate(kbs):
                    vht = vh0[:, kb, :] if kb < KB2 else vh1[:, kb - KB2, :]
                    nc.tensor.matmul(
                        psc[0:64, :SL],
                        vht,
                        pbt[:, SL * i:SL * (i + 1)],
                        start=(kb == 0), stop=(kb == KB - 1),
                    )

            # --- A pass: scores -> exp (+denom) -> normalize -> attn out
            for qb in range(QB):
                aps = psA.tile([128, ANW], F32, tag="A", name="aps")
                for w0 in range(0, ANW, 512):
                    ww = min(512, ANW - w0)
                    nc.tensor.matmul(
                        aps[:, w0:w0 + ww],
                        QT_sb[pi0:pi0 + 64, po, qb * 128:(qb + 1) * 128],
                        KT_sb[pi0:pi0 + 64, po, w0:w0 + ww],
                        start=True, stop=True,
                    )
                pa = pap.tile([128, ANW], F32, tag="pa", name="pa")
                nc.scalar.activation(
                    out=pa, in_=aps, func=AF.Exp, scale=scale,
                    accum_out=denoms[:, qb, 0:1],
                )
                rec = p2w.tile([128, QB], F32, tag="rec")
                nc.vector.reciprocal(rec[:, qb:qb + 1], denoms[:, qb, 0:1])
                HN = max(1, ANW // 1024)
                hw2 = ANW // HN
                for half in range(HN):
                    sl_ = slice(half * hw2, (half + 1) * hw2)
                    nc.vector.tensor_scalar(
                        pa[:, sl_], pa[:, sl_], rec[:, qb:qb + 1], None, ALU.mult
                    )
                    eng = nc.sync if (qb + half) % 2 == 0 else nc.gpsimd
                    eng.dma_start(
                        attn[h, qb * 128:(qb + 1) * 128, sl_], pa[:, sl_]
                    )

            # --- context^T scale by 1/denom (q on free dim) and place
            recq = p2w.tile([128, QB], F32, tag="rec2")
            nc.vector.reciprocal(recq[:, :], denoms[:, :, 0])
            # replicate recip 128-wide, transpose -> rows of per-q recip
            aps_t = psA.tile([128, ANW], F32, tag="A", name="apst")
            for c in range(QB):
                rec_rep = p2w.tile([128, 128], F32, tag="rec_rep")
                nc.vector.tensor_copy(
                    out=rec_rep[:, :],
                    in_=recq[:, c:c + 1].to_broadcast((128, 128)),
                )
                nc.tensor.transpose(
                    aps_t[:, 128 * c:128 * (c + 1)], rec_rep[:, :], ident,
                )
            Rrep = rrp.tile([128, SL], F32, tag="rrep", name="Rrep")
            nc.vector.tensor_copy(out=Rrep[0:64, :], in_=aps_t[0:64, :SL])
            if pi0 == 0:
                nc.vector.tensor_tensor(
                    ctxT_sb[0:64, po, :], psc[0:64, :SL], Rrep[0:64, :],
                    ALU.mult,
                )
            else:
                cstage = rrp.tile([128, SL], F32R, tag="cstage", name="cstage")
                nc.vector.tensor_tensor(
                    cstage[0:64, :], psc[0:64, :SL], Rrep[0:64, :], ALU.mult
                )
                nc.sync.dma_start(ctxT_sb[64:128, po, :], cstage[0:64, :])

        # ---------------- P3: out-proj + residual + LayerNorm ----------------
        with tc.tile_pool(name="p3w", bufs=3) as p3w, \
             tc.tile_pool(name="reps", bufs=1) as repsp:
            woT_sb = repsp.tile([128, KC, D], F32R)
            nc.gpsimd.dma_start(
                woT_sb[:, :, :], woT.ap().rearrange("(ko ki) d -> ki ko d", ki=128)
            )
            xr_sb = repsp.tile([128, QB, D], F32)
            nc.sync.dma_start(
                xr_sb[:, :, :], xrow.ap().rearrange("(qo qi) d -> qi qo d", qi=128)
            )

            def bcast_load(dr):
                t = repsp.tile([128, D], F32, tag=f"rep_{dr.name}",
                               name=f"rep_{dr.name}")
                src = dr[0:1, :]
                src_b = bass.AP(
                    tensor=src.tensor, offset=src.offset, ap=[[0, 128]] + src.ap[1:]
                )
                nc.gpsimd.dma_start(t[:, :], src_b)
                return t

            bo_rep = bcast_load(bo)
            g_rep = bcast_load(gam)
            b_rep = bcast_load(bet)

            fmax = int(np.gcd(512, D))
            nsub = D // fmax
            for m in range(QB):
                out_t = p3w.tile([128, D], F32, tag="out")
                # alternate between the (idle) attention psum regions so
                # m-block matmuls overlap the previous block's LayerNorm
                if m % 2 == 0:
                    preg = psA.tile([128, ANW], F32, tag="A", name="opsA")
                else:
                    preg = psB.tile([128, BMAX], F32, tag="B", name="opsB")
                for n0 in range(0, D, 512):
                    nw = min(512, D - n0)
                    ps = preg[:, n0 % 1024:n0 % 1024 + nw]
                    for k in range(KC):
                        nc.tensor.matmul(
                            ps, ctxT_sb[:, k, m * 128:(m + 1) * 128],
                            woT_sb[:, k, n0:n0 + nw],
                            start=(k == 0), stop=(k == KC - 1),
                        )
                    # out = psum + b_o  (fused copy+bias)
                    nc.vector.tensor_tensor(
                        out_t[:, n0:n0 + nw], ps, bo_rep[:, n0:n0 + nw], ALU.add
                    )
                # residual
                nc.vector.tensor_tensor(
                    out_t[:, :], out_t[:, :], xr_sb[:, m, :], ALU.add
                )
                # LayerNorm
                stats = p3w.tile([128, nsub, 6], F32, tag="st")
                for sg in range(nsub):
                    nc.vector.bn_stats(
                        out=stats[:, sg, :], in_=out_t[:, sg * fmax:(sg + 1) * fmax]
                    )
                mv = p3w.tile([128, 2], F32, tag="mv")
                nc.vector.bn_aggr(out=mv[:, :], in_=stats[:, :, :])
                rstd = p3w.tile([128, 1], F32, tag="rstd")
                nc.scalar.activation(
                    out=rstd[:, :], in_=mv[:, 1:2], func=AF.Sqrt, bias=eps_sb[:, :]
                )
                nc.vector.reciprocal(rstd[:, :], rstd[:, :])
                nc.vector.tensor_scalar(
                    out_t[:, :], out_t[:, :], mv[:, 0:1], rstd[:, :],
                    ALU.subtract, ALU.mult,
                )
                nc.vector.tensor_tensor(out_t[:, :], out_t[:, :], g_rep[:, :], ALU.mult)
                nc.vector.tensor_tensor(out_t[:, :], out_t[:, :], b_rep[:, :], ALU.add)
                nc.gpsimd.dma_start(y[m * 128:(m + 1) * 128, :], out_t[:, :])

    nc.finalize()
    return nc


_NC_CACHE = {}


def _get_nc(key=(D_MODEL, SEQ, N_HEADS)):
    if key not in _NC_CACHE:
        D, S, H = key
        _NC_CACHE[key] = build_nc(D=D, S=S, H=H)
    return _NC_CACHE[key]


def kernel(x, W_q, W_k, W_v, W_o, b_o, ln_gamma, ln_beta):
    x = np.asarray(x, dtype=np.float32)
    B, S, D = x.shape
    H = N_HEADS
    SL = S // 4
    nc = _get_nc((D, S, H))

    wqT = np.ascontiguousarray(np.asarray(W_q, np.float32).T)
    wkT = np.ascontiguousarray(np.asarray(W_k, np.float32).T)
    wvT = np.ascontiguousarray(np.asarray(W_v, np.float32).T)
    woT = np.ascontiguousarray(np.asarray(W_o, np.float32).T)
    bo = np.asarray(b_o, np.float32).reshape(1, D)
    gam = np.asarray(ln_gamma, np.float32).reshape(1, D)
    bet = np.asarray(ln_beta, np.float32).reshape(1, D)

    in_maps = []
    for c in range(N_CORES):
        b, t = c // 4, c % 4
        xT = np.ascontiguousarray(x[b].T)
        in_maps.append({
            "xT": xT,
            "xTq": np.ascontiguousarray(xT[:, SL * t:SL * (t + 1)]),
            "xrow": np.ascontiguousarray(x[b, SL * t:SL * (t + 1), :]),
            "wqT": wqT, "wkT": wkT, "wvT": wvT, "woT": woT,
            "bo": bo, "gam": gam, "bet": bet,
        })

    res = run_bass_kernel_spmd(nc, in_maps, core_ids=list(range(N_CORES)))

    y = np.empty((B, S, D), np.float32)
    attn = np.empty((B, H, S, S), np.float32)
    for c in range(N_CORES):
        b, t = c // 4, c % 4
        y[b, SL * t:SL * (t + 1), :] = res.results[c]["y"]
        attn[b, :, SL * t:SL * (t + 1), :] = res.results[c]["attn"]
    return y, attn
